# revision 71
# baseline (speedup 1.0000x reference)
"""Conv2d 3x3 (im2col GEMM) on 8 TRN2 NeuronCores.

Problem: x[16,64,112,112] (*) w[576,64] + b[64] -> out[16,64,112,112]
(3x3, stride 1, pad 1, NCHW, im2col patch order (c, kh, kw)).

Strategy
--------
Data-parallel over batch: 2 images per core, 8 cores, no collectives.

Per image, an implicit-GEMM formulation that needs only 3 full-width
matmuls per 448 outputs (vs 9 for naive per-tap GEMM):

  * x is staged in SBUF as z[128, F] in bf16: partitions 0:64 hold the
    image flattened row-major with a 113-element zero pad at each end
    ("zt"), partitions 64:128 hold the same data shifted left by one
    element ("zb"). Both halves arrive in ONE 128-partition DMA per row
    segment from a host-packed [128, HW+1] layout (partitions 64:128
    pre-shifted on the host), so staging has no on-device dependencies
    and compute overlaps the load. Segments are sized so the first
    matmul can start ~3us into the program.
  * For each kh in {0,1,2} one matmul with a block lhsT
        [[w(kh,1), w(kh,0)],
         [w(kh,2),    0   ]]
    accumulates into one 450-col psum chunk (4 output rows):
      psum[0:64,  j] += taps (kw=1 via zt) + (kw=2 via zb)  of out[s+j]
      psum[64:128,j] += tap  (kw=0 via zt)                  of out[s+j+1]
    Chunks are processed two-at-a-time in a [128, 1024] psum tile
    (one bank per chunk) so ACT/DVE post-ops run at 896 elements/op.
  * ACT adds bias to psum[64:128], DVE folds the two halves ->
    complete conv outputs (written as bf16).
  * Row-major flattening wraps at image-row boundaries. The kw=2 wrap
    terms (last column of each row) are eliminated for free: their zb
    source positions serve no other tap, so the host zeroes them in the
    packed layout. The kw=0 wrap terms (first column) read shared zt
    words, so tiny strided matmuls per half-image recompute exactly
    those terms and DVE subtracts them (bulk rows early, last rows in a
    small op so stores are never gated on a full-half subtraction).

bf16 everywhere on the wire (x, weights, output) with fp32 psum
accumulation: halves every DMA transfer vs fp32 - the DMA pool was the
binding resource of the fp32 version (54us of 74us span). Quantization
error ~4e-3 relative, well under the 2e-2 gate. All weights ship as ONE
DMA (packed [128, 576] tensor); 5 warm-up matmuls against a scratch
tile run under the load shadow each iteration so the PE p-state ramp
(0.65/1.2 -> 2.4 GHz after ~3us of continuous execution) is off the
critical path; edge subtractions are split bulk/tail and the final
store takes the low-latency HWDGE path so only a single 448-wide
ACT+DVE chain plus a tiny store trail the last matmul.
"""

import numpy as np
import ml_dtypes

import concourse.bacc as bacc
import concourse.mybir as mybir
import concourse.tile as tile
from concourse import bass_utils

# problem geometry (hardcoded per contract)
B, CIN, H, W = 16, 64, 112, 112
COUT = 64
NCORES = 8
IMGS = B // NCORES  # images per core

HW = H * W                     # 12544
ZOFF = W + 1                   # lead zero pad: 1 + one full pad row
F = ZOFF + HW + ZOFF           # z free size per image (12770)
ROWS_PER_CHUNK = 4
CHUNK = ROWS_PER_CHUNK * W     # 448 outputs per psum chunk
MMW = CHUNK + 2                # 450: matmul moving width
NCHUNK = H // ROWS_PER_CHUNK   # 28
NPAIR = NCHUNK // 2            # 14 chunk pairs per image
PAIRS_PER_HALF = NPAIR // 2    # 7
HALF_ROWS = H // 2             # 56
HALF = HALF_ROWS * W           # 6272 outputs per store half
# z load segments (z-flat cut points): output-row coverage + 2-row halo.
# Small first segment so the first matmul can start ~2.5us in.
SEG_ROWS = [6, 12, 22, 40, 66, 92, H]
SEG_END = [ZOFF + min(r + 2, H) * W for r in SEG_ROWS]
# z alloc pad so the (sliced-then-strided) correction rhs views stay in
# bounds; the strided APs themselves never read past F.
F_ALLOC = F + 111

f32 = mybir.dt.float32
bf16 = mybir.dt.bfloat16
u16 = mybir.dt.uint16

_cache = {}

bfloat16 = ml_dtypes.bfloat16


def _prep_weights(weight):
    """Host-side: pack block lhsT matrices + kw=2 slices into one
    [128, 3*128 + 3*64] bf16 tensor (single DMA)."""
    w_r = np.asarray(weight, np.float32).astype(bfloat16)
    w_r = w_r.reshape(CIN, 3, 3, COUT)  # [c, kh, kw, m]
    wt = np.zeros((128, 3 * 128 + 3 * 64), bfloat16)
    for kh in range(3):
        blk = wt[:, kh * 128: (kh + 1) * 128]
        blk[0:64, 0:64] = w_r[:, kh, 1, :]
        blk[0:64, 64:128] = w_r[:, kh, 0, :]
        blk[64:128, 0:64] = w_r[:, kh, 2, :]
        wt[0:64, 384 + kh * 64: 384 + (kh + 1) * 64] = w_r[:, kh, 2, :]
    return wt


def _build(repeat=None):
    nc = bacc.Bacc("TRN2", target_bir_lowering=False, debug=False,
                   num_devices=NCORES)

    # xz: host-packed staging layout [128, HW+1] per image.
    # partitions 0:64  = [0, x[c, :]]      (zt: lead slot 0, then image)
    # partitions 64:128 = [x[c, :], 0]     (zb: image shifted left by one)
    # One 128-partition DMA per segment fills both the zt and zb halves
    # of z - the zb content is bit-identical to the old SBUF shift copy.
    xz_d = nc.dram_tensor("xz", (IMGS, 128, HW + 1), bf16,
                          kind="ExternalInput")
    wt_d = nc.dram_tensor("wts", (128, 576), bf16, kind="ExternalInput")
    b_d = nc.dram_tensor("bias", (COUT,), f32, kind="ExternalInput")
    o_d = nc.dram_tensor("out", (IMGS, COUT, H, W), bf16,
                         kind="ExternalOutput")

    xzv = xz_d.ap()
    ov = o_d.ap().rearrange("b c h w -> b c (h w)")

    with tile.TileContext(nc) as tc:
        with (
            tc.tile_pool(name="wpool", bufs=1) as wpool,
            tc.tile_pool(name="zpool", bufs=1) as zpool,
            tc.tile_pool(name="opool", bufs=3) as opool,
            tc.tile_pool(name="tpool", bufs=4) as tpool,
            tc.tile_pool(name="ppool", bufs=4, space="PSUM") as ppool,
        ):
            # --- weights / bias staging: SP ring, ahead of the x loads
            # (tiny transfers; wt gates the PE warm-up matmuls) ---
            # wt first on SP (tiny, and it gates the first real matmul);
            # bias on the ACT ring behind its activation-table load
            # (needed only ~7us in)
            wt = wpool.tile([128, 576], bf16, name="wt", tag="wt")
            nc.sync.dma_start(wt[:, :], wt_d.ap())
            bias = wpool.tile([COUT, 1], f32)
            nc.scalar.dma_start(
                bias[:, :], b_d.ap().rearrange("(c one) -> c one", one=1))
            # scratch lhsT for the PE warm-up matmuls: contents are
            # irrelevant (results never read), a memset just gives it a
            # producer so the warm-ups depend on nothing slow
            wlhs = wpool.tile([128, 512], bf16, name="wlhs", tag="wlhs")
            nc.gpsimd.memset(wlhs[:, :].bitcast(u16), 0)

            lhs = [wt[:, kh * 128: (kh + 1) * 128] for kh in range(3)]

            # Persistent z tiles (one per image, no rotation): the zero
            # pads are invariant, so memset them ONCE here instead of per
            # For_i iteration - per-iter pad memsets were head-of-line
            # blocking the next iteration's staging DMAs.
            # Pad regions are DISJOINT from every load destination (the
            # xz layout itself writes the correct boundary values at cols
            # ZOFF-1 and ZOFF+HW-1), so loads never carry a WAR edge
            # against these one-time memsets.
            zs = [zpool.tile([128, F_ALLOC], bf16, name=f"z{i}", tag=f"z{i}")
                  for i in range(IMGS)]
            for z in zs:
                nc.vector.memset(z[:, 0:ZOFF - 1].bitcast(u16), 0)
                nc.vector.memset(z[:, ZOFF + HW: F].bitcast(u16), 0)

            import contextlib
            loop_cm = (
                tc.For_i(0, repeat, 1)
                if repeat is not None else contextlib.nullcontext()
            )
            with loop_cm:
              # PE p-state warm-up: the tensor engine only reaches max
              # clock after ~3us of continuous execution. While the first
              # x segments are in flight the PE is idle anyway, so burn
              # that shadow on dummy matmuls against the (resident)
              # weights tile. Results land in the correction psum tile
              # and are never read.
              warm = ppool.tile([128, 1024], f32, name="pcw", tag="ps")
              for wi in range(7):
                  nc.tensor.matmul(warm[0:64, 0:512], wlhs[:, 0:64],
                                   wlhs[:, 0:512], start=True, stop=True)

              def prep(img):
                  # One 128-partition DMA per segment fills zt and zb at
                  # once from the host-packed xz layout. All on the SP
                  # ring; stores live on the SWDGE ring so a pending
                  # store never blocks staging.
                  z = zs[img]
                  prev = ZOFF - 1
                  for end in SEG_END:
                      nc.sync.dma_start(
                          z[:, prev:end],
                          xzv[img][:, prev - (ZOFF - 1): end - (ZOFF - 1)])
                      prev = end

              for img in range(IMGS):
                  prep(img)
                  z = zs[img]

                  for half in range(2):
                      og = opool.tile([COUT, HALF], bf16, name="og", tag="og")
                      ogr = og[:, :].rearrange("q (r w) -> q r w", w=W)
                      pc1 = None
                      BROWS = (PAIRS_PER_HALF - 2) * 2 * ROWS_PER_CHUNK  # 40
                      tail_half = (img == IMGS - 1 and half == 1)

                      for pp in range(PAIRS_PER_HALF):
                          p = half * PAIRS_PER_HALF + pp
                          if tail_half and pp == PAIRS_PER_HALF - 1:
                              # program's final pair: two single-chunk
                              # psum groups, so chunk 26's whole post-op
                              # chain (and the rows 40:52 sub+store) runs
                              # under chunk 27's matmuls - only one
                              # 448-wide ACT+DVE chain plus a 4-row store
                              # trails the last matmul
                              MIDR = HALF_ROWS - ROWS_PER_CHUNK  # 52
                              for k in range(2):
                                  psk = ppool.tile([128, 1024], f32,
                                                   name="pss", tag="ps")
                                  y0 = (2 * p + k) * ROWS_PER_CHUNK
                                  for kh in range(3):
                                      a = (y0 + kh) * W
                                      nc.tensor.matmul(
                                          psk[:, 0:MMW], lhs[kh],
                                          z[:, a: a + MMW],
                                          start=(kh == 0), stop=(kh == 2))
                                  tbk = tpool.tile([COUT, CHUNK], f32,
                                                   name="tbs", tag="tbs")
                                  nc.scalar.add(tbk[:, :],
                                                psk[64:128, 0:CHUNK],
                                                bias[:, :])
                                  ogk = og[:, (2 * pp + k) * CHUNK:
                                           (2 * pp + k + 1) * CHUNK]
                                  nc.vector.tensor_add(
                                      ogk, psk[0:64, 1: CHUNK + 1],
                                      tbk[:, :])
                                  if k == 0:
                                      nc.vector.tensor_sub(
                                          ogr[:, BROWS:MIDR, 0],
                                          ogr[:, BROWS:MIDR, 0],
                                          pc1[:, BROWS:MIDR])
                                      nc.gpsimd.dma_start(
                                          ov[img, :,
                                             half * HALF + BROWS * W:
                                             half * HALF + MIDR * W],
                                          og[:, BROWS * W: MIDR * W])
                              nc.vector.tensor_sub(
                                  ogr[:, MIDR:HALF_ROWS, 0],
                                  ogr[:, MIDR:HALF_ROWS, 0],
                                  pc1[:, MIDR:HALF_ROWS])
                              nc.sync.dma_start(
                                  ov[img, :, half * HALF + MIDR * W:
                                     (half + 1) * HALF],
                                  og[:, MIDR * W: HALF])
                              continue
                          ps = ppool.tile([128, 1024], f32, name="ps",
                                          tag="ps")
                          for k in range(2):
                            for kh in range(3):
                              y0 = (2 * p + k) * ROWS_PER_CHUNK
                              a = (y0 + kh) * W
                              nc.tensor.matmul(
                                  ps[:, 512 * k: 512 * k + MMW],
                                  lhs[kh],
                                  z[:, a: a + MMW],
                                  start=(kh == 0),
                                  stop=(kh == 2),
                              )
                          psv = ps[:, :].rearrange("q (a b) -> q a b", b=512)
                          tb = tpool.tile([COUT, 2 * CHUNK], f32, name="tb",
                                          tag="tb")
                          tbv = tb[:, :].rearrange("q (a b) -> q a b",
                                                   b=CHUNK)
                          ogv = og[:, pp * 2 * CHUNK: (pp + 1) * 2 * CHUNK
                                   ].rearrange("q (a b) -> q a b", b=CHUNK)
                          psa = ps[:, :].rearrange(
                              "q (a b) -> q a b", b=512)[0:64, :, 1: CHUNK + 1]
                          if tail_half and pp >= PAIRS_PER_HALF - 2:
                              # final two pairs of the program: split the
                              # bias-add/fold per chunk - chunk 0's ACT
                              # runs under chunk 1's matmuls, so only one
                              # 448-wide ACT+DVE chain trails the last
                              # matmul
                              for k in range(2):
                                  nc.scalar.add(tbv[:, k, :],
                                                psv[64:128, k, 0:CHUNK],
                                                bias[:, :])
                              for k in range(2):
                                  nc.vector.tensor_add(
                                      ogv[:, k, :], psa[:, k, :],
                                      tbv[:, k, :])
                          else:
                              nc.scalar.add(tbv, psv[64:128, :, 0:CHUNK],
                                            bias[:, :])
                              nc.vector.tensor_add(ogv, psa, tbv)
                          if pp == PAIRS_PER_HALF - 3:
                              # --- kw=0 edge correction, rows
                              # 56*half..+55, in a dedicated psum pool so
                              # no main-matmul rotation slot ever waits
                              # on its late sub reads. Emitted after this
                              # pair's matmuls (it needs the half's full
                              # z rows). The kw=2 wraps need no
                              # correction: their zb source positions are
                              # zeroed host-side in the packed xz layout.
                              pct = ppool.tile([128, 1024], f32, name="pc",
                                               tag="ps")
                              pc1 = pct[0:64, 0:HALF_ROWS]
                              for kh in range(3):
                                  a = (HALF_ROWS * half + kh) * W
                                  rhs = z[0:64, a: a + HALF_ROWS * W
                                          ].rearrange(
                                      "q (r w) -> q r w", w=W)[:, :, 0]
                                  nc.tensor.matmul(
                                      pc1, lhs[kh][0:64, 64:128], rhs,
                                      start=(kh == 0), stop=(kh == 2))
                              # bulk edge subs (rows covered by pairs
                              # 0..4) run while the last pairs compute
                              nc.vector.tensor_sub(
                                  ogr[:, 0:BROWS, 0], ogr[:, 0:BROWS, 0],
                                  pc1[:, 0:BROWS])
                              # rows 0..47 are final after the bulk subs:
                              # store them early so the tail store is tiny
                              nc.gpsimd.dma_start(
                                  ov[img, :,
                                     half * HALF: half * HALF + BROWS * W],
                                  og[:, 0: BROWS * W])

                      if not tail_half:
                          # last pairs' edge sub + store (tail_half
                          # handles these inside its final-pair branch)
                          nc.vector.tensor_sub(
                              ogr[:, BROWS:HALF_ROWS, 0],
                              ogr[:, BROWS:HALF_ROWS, 0],
                              pc1[:, BROWS:HALF_ROWS])
                          nc.gpsimd.dma_start(
                              ov[img, :, half * HALF + BROWS * W:
                                 (half + 1) * HALF],
                              og[:, BROWS * W: HALF])

    nc.compile()
    return nc


def prep_in_maps(x, weight, bias):
    x_bf = np.ascontiguousarray(x, np.float32).astype(bfloat16)
    x_bf = x_bf.reshape(B, CIN, HW)
    xz = np.zeros((B, 128, HW + 1), bfloat16)
    xz[:, 0:64, 1:] = x_bf
    xz[:, 64:128, :HW] = x_bf
    # zb wrap positions (cols kW, k=0..111; the HW tail slot is already
    # zero): these feed ONLY the kw=2 taps of column W-1 outputs, whose
    # true value is the zero pad. Zeroing them here replaces the whole
    # pc2 device-side correction (12 matmuls + 4 subs per iteration).
    xz[:, 64:128, 0:HW:W] = 0
    wt_np = _prep_weights(weight)
    b_np = np.ascontiguousarray(bias, dtype=np.float32)
    return [
        {
            "xz": np.ascontiguousarray(xz[i * IMGS: (i + 1) * IMGS]),
            "wts": wt_np,
            "bias": b_np,
        }
        for i in range(NCORES)
    ]


def kernel(x: np.ndarray, weight: np.ndarray, bias: np.ndarray,
           **_ignored) -> np.ndarray:
    if "nc" not in _cache:
        _cache["nc"] = _build()
    nc = _cache["nc"]

    in_maps = prep_in_maps(x, weight, bias)
    res = bass_utils.run_bass_kernel_spmd(
        nc, in_maps, core_ids=list(range(NCORES)))
    out = np.concatenate(
        [np.asarray(r["out"]).astype(np.float32) for r in res.results],
        axis=0)
    return out.reshape(B, COUT, H, W)
